# Initial kernel scaffold
#
"""Causal sliding-window attention (B=2, T=2048, D=1024, H=16, W=512) on 8 trn2 cores.

Sequence-parallel sharding: each core owns 512 consecutive tokens of one batch
and recomputes the 512-token halo k/v locally (no collectives). All compute is
feature-major (tokens on the matmul free dim) in float32r:

  xT -> qT/kT (feature-major), v (token-major, with a ones column per head)
  scoresT[keys, q] = kT_h.T-free matmul -> exp on ACT (bias kills chunk-0 halo)
  band masks: 0/1 multiplies on the two diagonal key-tiles per query window
  attV: v-stationary matmul; the ones column yields softmax sums as a psum row
  normalize: DVE reciprocal + partition-broadcast DMA + psum-evicting multiply
  outT = wo-stationary matmul over attT; host transposes/concats core outputs.
"""
import sys

sys.path.insert(0, "/opt/trn_rl_repo")

import numpy as np

B, T, D = 2, 2048, 1024
H, HD, W = 16, 64, 512
NCORES = 8
CHUNK = 512  # own tokens per core
TOK = 2 * CHUNK  # halo + own
NKD = D // 128  # 8 contraction tiles
SCALE = HD ** -0.5

# query-window [qlo, qhi) per key-tile kb, padded to >=256 cols for fp32r rate
QRANGE = []
for kb in range(8):
    qlo = max(0, 128 * kb - 512)
    qhi = min(512, 128 * kb + 128)
    if qhi - qlo < 256:
        qlo, qhi = (0, 256) if qlo == 0 else (256, 512)
    QRANGE.append((qlo, qhi))

_BUILT = None


def _build():
    import concourse.bass as bass
    import concourse.tile as tile
    from concourse import mybir, bacc

    f32 = mybir.dt.float32
    f32r = mybir.dt.float32r

    nc = bacc.Bacc("TRN2", target_bir_lowering=False, debug=False,
                   num_devices=NCORES)
    xT = nc.dram_tensor("xT", [D, TOK], f32r, kind="ExternalInput")
    wq = nc.dram_tensor("wq", [D, D], f32r, kind="ExternalInput")
    wk = nc.dram_tensor("wk", [D, D], f32r, kind="ExternalInput")
    wv = nc.dram_tensor("wv", [D, D], f32r, kind="ExternalInput")
    wo = nc.dram_tensor("wo", [D, D], f32r, kind="ExternalInput")
    # [:, 0, :] = strict-lower-tri (j>q edge), [:, 1, :] = upper-incl
    # (j<=q+W edge), [:, 2, :] = zeros (kills fully-invalid padded cols)
    mask = nc.dram_tensor("mask", [128, 3, 128], f32, kind="ExternalInput")
    vones = nc.dram_tensor("vones", [128, H], f32r, kind="ExternalInput")
    kbias = nc.dram_tensor("kbias", [128, NKD], f32, kind="ExternalInput")
    outT = nc.dram_tensor("outT", [D, CHUNK], f32, kind="ExternalOutput")
    # per-head softmax-recip row, bounced through DRAM to broadcast across
    # partitions (SBUF DMA sources cannot have a zero partition step)
    rscratch = nc.dram_tensor("rscratch", [H, CHUNK], f32, kind="Internal")

    xT_r = xT.rearrange("(kd p) t -> kd p t", p=128)
    w_r = {n: w.rearrange("(kd p) c -> kd p c", p=128)
           for n, w in (("wq", wq), ("wk", wk), ("wv", wv))}
    # wo reshaped so every head's 64 contraction rows sit at partition base 0
    # (matmul needs lhsT and rhs on the same base; attT halves live at 0..63)
    wo_r = wo.rearrange("(hh d) e -> d hh e", hh=H)

    with tile.TileContext(nc) as tc:
        with tc.tile_pool(name="const", bufs=1) as constp, \
             tc.tile_pool(name="qkv", bufs=1) as qkvp, \
             tc.tile_pool(name="ps_mm", bufs=2, space="PSUM") as ps_mm:

            mask_sb = constp.tile([128, 3, 128], f32)
            nc.sync.dma_start(out=mask_sb, in_=mask[:, :, :])
            kbias_sb = constp.tile([128, NKD], f32)
            nc.sync.dma_start(out=kbias_sb, in_=kbias[:, :])

            # ---- persistent qkv buffers (feature-major q/k, token-major v)
            qT_sb = qkvp.tile([128, NKD, CHUNK], f32r)   # q dims x own tokens
            kT_sb = qkvp.tile([128, NKD, TOK], f32r)     # k dims x keys
            # v: per key-tile and head, 65 stationary columns: cols [0:64] = v,
            # col 64 = ones -> attV psum rows 0..63 = att, row 64 = softmax sum
            v_sb = qkvp.tile([128, NKD, H // 2, 2, 65], f32r)

            with tc.tile_pool(name="wts", bufs=2) as wpool, \
                 tc.tile_pool(name="xp", bufs=1) as xp:
                w_tiles = {}
                for wn in ("wq", "wk", "wv"):
                    w_tiles[wn] = [
                        wpool.tile([128, D], f32r, tag=f"w{kd}",
                                   name=f"{wn}_{kd}")
                        for kd in range(NKD)]
                    for kd in range(NKD):
                        nc.sync.dma_start(out=w_tiles[wn][kd], in_=w_r[wn][kd])
                x_tiles = [xp.tile([128, TOK], f32r, tag=f"x{kd}",
                                   name=f"x_{kd}")
                           for kd in range(NKD)]
                for kd in range(NKD):
                    nc.sync.dma_start(out=x_tiles[kd], in_=xT_r[kd])

                # ---- q projection: qT[co] = sum_kd wq[kd,co].T @ xT[kd, own]
                for co in range(NKD):
                    ps = ps_mm.tile([128, CHUNK], f32)
                    for kd in range(NKD):
                        nc.tensor.matmul(
                            ps[:],
                            w_tiles["wq"][kd][:, co * 128:(co + 1) * 128],
                            x_tiles[kd][:, CHUNK:TOK],
                            start=(kd == 0), stop=(kd == NKD - 1))
                    nc.scalar.copy(qT_sb[:, co, :], ps[:])

                # ---- k projection over all TOK keys
                for co in range(NKD):
                    for th in range(2):
                        ps = ps_mm.tile([128, CHUNK], f32)
                        for kd in range(NKD):
                            nc.tensor.matmul(
                                ps[:],
                                w_tiles["wk"][kd][:, co * 128:(co + 1) * 128],
                                x_tiles[kd][:, th * CHUNK:(th + 1) * CHUNK],
                                start=(kd == 0), stop=(kd == NKD - 1))
                        nc.scalar.copy(kT_sb[:, co, th * CHUNK:(th + 1) * CHUNK],
                                       ps[:])

                # ---- v projection, token-major: v[tt] = xT[:,tt].T @ wv
                for tt in range(NKD):
                    for cv in range(2):
                        ps = ps_mm.tile([128, CHUNK], f32)
                        for kd in range(NKD):
                            nc.tensor.matmul(
                                ps[:],
                                x_tiles[kd][:, tt * 128:(tt + 1) * 128],
                                w_tiles["wv"][kd][:, cv * CHUNK:(cv + 1) * CHUNK],
                                start=(kd == 0), stop=(kd == NKD - 1))
                        # scatter 8 heads (cols of 64) into the 65-col slots
                        ps4 = ps[:].rearrange("p (g par d) -> p g par d",
                                              par=2, d=HD)
                        g0 = cv * 4
                        nc.scalar.copy(
                            v_sb[:, tt, g0:g0 + 4, 0, 0:HD], ps4[:, :, 0, :])
                        nc.scalar.copy(
                            v_sb[:, tt, g0:g0 + 4, 1, 0:HD], ps4[:, :, 1, :])
                # ones column of every stationary (memset can't write f32r)
                for tt in range(NKD):
                    nc.sync.dma_start(
                        out=v_sb[:, tt, :, :, HD:HD + 1], in_=vones[:, :])

            # ---- attention + output projection
            with tc.tile_pool(name="attb", bufs=1) as attbp, \
                 tc.tile_pool(name="pt", bufs=3) as ptp, \
                 tc.tile_pool(name="nrm", bufs=2) as nrmp, \
                 tc.tile_pool(name="oev", bufs=1) as oevp, \
                 tc.tile_pool(name="ps_sc", bufs=3, space="PSUM") as ps_sc, \
                 tc.tile_pool(name="ps_at", bufs=3, space="PSUM") as ps_at:

                # att features split by head parity so every head's psum rows
                # (0..63, sums at 64) evict to partitions 0..63
                attT_e = attbp.tile([64, NKD, CHUNK], f32r)
                attT_o = attbp.tile([64, NKD, CHUNK], f32r)
                attT_sb = (attT_e, attT_o)

                # wo with all 16 head-row-groups at partition base 0; loads
                # overlap the attention phase (slot reuses freed x/w space)
                wo_sb = attbp.tile([64, H, D], f32r)
                for g in range(4):
                    nc.sync.dma_start(out=wo_sb[:, 4 * g:4 * (g + 1), :],
                                      in_=wo_r[:, 4 * g:4 * (g + 1), :])

                KB_ORDER = [3, 4, 0, 1, 2, 5, 6, 7]  # first covers q[0:512)
                for h in range(H):
                    hp, po = h // 2, (h % 2) * 64
                    att_ps = ps_at.tile([128, CHUNK], f32)
                    for i, kb in enumerate(KB_ORDER):
                        qlo, qhi = QRANGE[kb]
                        wdt = qhi - qlo
                        sc_ps = ps_sc.tile([128, CHUNK], f32, tag="sc")
                        nc.tensor.matmul(
                            sc_ps[:, 0:wdt],
                            kT_sb[po:po + 64, hp, kb * 128:(kb + 1) * 128],
                            qT_sb[po:po + 64, hp, qlo:qhi],
                            start=True, stop=True)
                        pt = ptp.tile([128, CHUNK], f32r, tag="pt")
                        nc.scalar.activation(
                            pt[:, 0:wdt], sc_ps[:, 0:wdt],
                            mybir.ActivationFunctionType.Exp,
                            bias=kbias_sb[:, kb:kb + 1], scale=SCALE)
                        # band-edge masking on the diagonal 128-col block
                        if kb <= 3:
                            dlo = 128 * kb - qlo
                            nc.vector.tensor_mul(
                                pt[:, dlo:dlo + 128], pt[:, dlo:dlo + 128],
                                mask_sb[:, 0, :])
                            if kb == 0:  # padded cols are fully invalid
                                nc.vector.tensor_mul(
                                    pt[:, 128:256], pt[:, 128:256],
                                    mask_sb[:, 2, :])
                        else:
                            dlo = 128 * (kb - 4) - qlo
                            nc.vector.tensor_mul(
                                pt[:, dlo:dlo + 128], pt[:, dlo:dlo + 128],
                                mask_sb[:, 1, :])
                            if kb == 7:
                                nc.vector.tensor_mul(
                                    pt[:, 0:128], pt[:, 0:128],
                                    mask_sb[:, 2, :])
                        nc.tensor.matmul(
                            att_ps[0:65, qlo:qhi],
                            v_sb[:, kb, hp, h % 2, :],
                            pt[:, 0:wdt],
                            start=(i == 0), stop=(i == len(KB_ORDER) - 1))
                    # normalize: recip of the sums row, broadcast, multiply
                    recip = nrmp.tile([128, CHUNK], f32, tag="recip")
                    nc.vector.reciprocal(recip[64:65, :], att_ps[64:65, :])
                    bc = nrmp.tile([128, CHUNK], f32, tag="bc")
                    nc.sync.dma_start(out=rscratch[h:h + 1, :],
                                      in_=recip[64:65, :])
                    bcast_src = bass.AP(
                        tensor=rscratch, offset=h * CHUNK,
                        ap=[[0, 64], [1, CHUNK]])
                    nc.gpsimd.dma_start(out=bc[0:64, :], in_=bcast_src)
                    nc.vector.tensor_mul(
                        attT_sb[h % 2][0:64, hp, :],
                        att_ps[0:64, :],
                        bc[0:64, :])

                # ---- output projection: 16 K=64 half-matmuls per output tile
                for eo in range(NKD):
                    ps = ps_mm.tile([128, CHUNK], f32)
                    for hh in range(H):
                        nc.tensor.matmul(
                            ps[:],
                            wo_sb[0:64, hh, eo * 128:(eo + 1) * 128],
                            attT_sb[hh % 2][0:64, hh // 2, :],
                            start=(hh == 0), stop=(hh == H - 1))
                    ot = oevp.tile([128, CHUNK], f32, tag="ot")
                    nc.scalar.copy(ot[:], ps[:])
                    nc.sync.dma_start(out=outT[eo * 128:(eo + 1) * 128, :],
                                      in_=ot[:])

    nc.compile()
    return nc


def _host_inputs(x, w_qkv, w_out):
    x = np.ascontiguousarray(np.asarray(x, dtype=np.float32))
    w_qkv = np.ascontiguousarray(np.asarray(w_qkv, dtype=np.float32))
    w_out = np.ascontiguousarray(np.asarray(w_out, dtype=np.float32))

    wq = np.ascontiguousarray(w_qkv[:, 0:D])
    wk = np.ascontiguousarray(w_qkv[:, D:2 * D])
    wv = np.ascontiguousarray(w_qkv[:, 2 * D:3 * D])

    r = np.arange(128)[:, None]
    c = np.arange(128)[None, :]
    mask = np.zeros((128, 3, 128), dtype=np.float32)
    mask[:, 0, :] = (r > c).astype(np.float32)
    mask[:, 1, :] = (r <= c).astype(np.float32)
    vones = np.ones((128, H), dtype=np.float32)

    in_maps = []
    for core in range(NCORES):
        b, qc = divmod(core, 4)
        q0 = qc * CHUNK
        xa = np.zeros((TOK, D), dtype=np.float32)
        lo = max(0, q0 - CHUNK)
        xa[CHUNK - (q0 - lo):] = x[b, lo:q0 + CHUNK]
        kb_bias = np.zeros((128, NKD), dtype=np.float32)
        if qc == 0:
            kb_bias[:, 0:4] = -250.0
        in_maps.append({
            "xT": np.ascontiguousarray(xa.T),
            "wq": wq, "wk": wk, "wv": wv, "wo": w_out,
            "mask": mask, "kbias": kb_bias, "vones": vones,
        })
    return in_maps


def kernel(x, w_qkv, w_out):
    global _BUILT
    if _BUILT is None:
        _BUILT = _build()
    from concourse.bass_utils import run_bass_kernel_spmd

    in_maps = _host_inputs(x, w_qkv, w_out)
    res = run_bass_kernel_spmd(_BUILT, in_maps, core_ids=list(range(NCORES)))
    out = np.empty((B, T, D), dtype=np.float32)
    for core in range(NCORES):
        b, qc = divmod(core, 4)
        out[b, qc * CHUNK:(qc + 1) * CHUNK, :] = res.results[core]["outT"].T
    return out



# revision 11
# speedup vs baseline: 1.1656x; 1.1656x over previous
"""Causal sliding-window attention (B=2, T=2048, D=1024, H=16, W=512) on 8 trn2 cores.

Sequence-parallel sharding: each core owns 512 consecutive tokens of one batch
and recomputes the 512-token halo k/v locally (no collectives). All compute is
feature-major (tokens on the matmul free dim) in float32r.

v2 layout (vs baseline): engine-balanced pipeline.
  - startup: 2-chunk bulk DMAs per tensor + kd-outer projection chains so the
    PE starts ~4us in (HWDGE issue is 625ns/DMA; many small DMAs stall start).
  - attention: the 8 key-tiles per head are packed into 4 psum score groups
    (A=kb1|kb0, B=kb3|kb2, C=kb4|kb5, D=Z|kb7|kb6) with query-aligned column
    layouts; one exp per group (4 ACT instrs/head), one fused band-mask
    multiply per group (strided/repeated APs, 4 DVE instrs/head), attV streams
    always >=256 wide at full fp32r rate; padded windows read permanently
    zeroed pt columns instead of being zeroed per-head.
  - softmax: ones-column in the attV stationary gives sums in psum row 64;
    DVE reciprocal -> gpsimd partition_broadcast (no DRAM bounce) -> DVE mul.
  - out-proj: heads paired to K=128 contraction (8 matmuls per output tile);
    odd heads' normalized attT is partition-shifted 0:64 -> 64:128 by a tiny
    SBUF->SBUF DMA so each pair's wo rows are a contiguous 128-row block.
"""
import sys

sys.path.insert(0, "/opt/trn_rl_repo")

import numpy as np

B, T, D = 2, 2048, 1024
H, HD, W = 16, 64, 512
NCORES = 8
CHUNK = 512  # own tokens per core
TOK = 2 * CHUNK  # halo + own
NKD = D // 128  # 8 contraction tiles
SCALE = HD ** -0.5

_BUILT = None


def _build():
    import concourse.bass as bass
    import concourse.tile as tile
    from concourse import mybir, bacc

    f32 = mybir.dt.float32
    f32r = mybir.dt.float32r

    nc = bacc.Bacc("TRN2", target_bir_lowering=False, debug=False,
                   num_devices=NCORES)
    xT = nc.dram_tensor("xT", [D, TOK], f32r, kind="ExternalInput")
    wq = nc.dram_tensor("wq", [D, D], f32r, kind="ExternalInput")
    wk = nc.dram_tensor("wk", [D, D], f32r, kind="ExternalInput")
    wv = nc.dram_tensor("wv", [D, D], f32r, kind="ExternalInput")
    wo = nc.dram_tensor("wo", [D, D], f32r, kind="ExternalInput")
    # [:, 0, :] = strict-lower-tri (halo diag blocks), [:, 1, :] = upper-incl
    mask = nc.dram_tensor("mask", [128, 2, 128], f32, kind="ExternalInput")
    # exp bias per score group (A,B = halo: -250 on chunk-0 cores; C,D = 0)
    kbias = nc.dram_tensor("kbias", [128, 4], f32, kind="ExternalInput")
    vones = nc.dram_tensor("vones", [128, NKD, H], f32r, kind="ExternalInput")
    zpad = nc.dram_tensor("zpad", [128, 128], f32r, kind="ExternalInput")
    outT = nc.dram_tensor("outT", [D, CHUNK], f32, kind="ExternalOutput")

    xT_r = xT.rearrange("(kd p) t -> kd p t", p=128)
    w_r = {n: w.rearrange("(kd p) c -> kd p c", p=128)
           for n, w in (("wq", wq), ("wk", wk), ("wv", wv))}
    # wo with head-pair rows contiguous: partition p of pair hp = wo row
    # 128*hp + p (head 2hp dims at p<64, head 2hp+1 dims at p>=64)
    wo_r = wo.rearrange("(hp p) e -> p hp e", p=128)

    with tile.TileContext(nc) as tc:
        with tc.tile_pool(name="const", bufs=1) as constp, \
             tc.tile_pool(name="qkv", bufs=1) as qkvp:

            mask_sb = constp.tile([128, 2, 128], f32)
            nc.sync.dma_start(out=mask_sb, in_=mask[:, :, :])
            kbias_sb = constp.tile([128, 4], f32)
            nc.sync.dma_start(out=kbias_sb, in_=kbias[:, :])

            # ---- persistent qkv buffers (feature-major q/k, token-major v)
            qT_sb = qkvp.tile([128, NKD, CHUNK], f32r)   # q dims x own tokens
            kT_sb = qkvp.tile([128, NKD, TOK], f32r)     # k dims x keys
            # v: per key-tile and head, 65 stationary columns: cols [0:64] = v,
            # col 64 = ones -> attV psum rows 0..63 = att, row 64 = softmax sum
            v_sb = qkvp.tile([128, NKD, H // 2, 2, 65], f32r)

            with tc.tile_pool(name="wx", bufs=1) as wxp, \
                 tc.tile_pool(name="qkv_ps", bufs=1, space="PSUM") as qps:
                # x: own half first (gates the first matmul), halo second
                x_own = wxp.tile([128, NKD, CHUNK], f32r)
                x_halo = wxp.tile([128, NKD, CHUNK], f32r)
                # wv reuses wq's buffers (wq is dead once q-proj finishes);
                # its tiles are allocated after the q-proj loop below.
                wtags = {"wq": ("wbA", "wbB"), "wk": ("wbC", "wbD"),
                         "wv": ("wbA", "wbB")}
                w_sb = {}
                for wn in ("wq", "wk"):
                    w_sb[wn] = [wxp.tile([128, 4, D], f32r,
                                         name=f"{wn}_{half}",
                                         tag=wtags[wn][half])
                                for half in range(2)]
                nc.sync.dma_start(
                    out=x_own,
                    in_=xT_r[:, :, CHUNK:TOK].rearrange("kd p t -> p kd t"))
                for half in range(2):
                    nc.sync.dma_start(
                        out=w_sb["wq"][half],
                        in_=w_r["wq"][4 * half:4 * half + 4]
                        .rearrange("kd p c -> p kd c"))
                nc.sync.dma_start(
                    out=x_halo,
                    in_=xT_r[:, :, 0:CHUNK].rearrange("kd p t -> p kd t"))
                for half in range(2):
                    nc.sync.dma_start(
                        out=w_sb["wk"][half],
                        in_=w_r["wk"][4 * half:4 * half + 4]
                        .rearrange("kd p c -> p kd c"))
                nc.sync.dma_start(out=v_sb[:, :, :, :, HD:HD + 1],
                                  in_=vones[:, :, :])

                def wt(wn, kd):
                    return w_sb[wn][kd // 4][:, kd % 4, :]

                # ---- q projection, kd-outer: 8 parallel psum chains
                ps_q = [qps.tile([128, CHUNK], f32, name=f"psq{co}",
                                 tag=f"ps{co}") for co in range(NKD)]
                for kd in range(NKD):
                    for co in range(NKD):
                        nc.tensor.matmul(
                            ps_q[co][:],
                            wt("wq", kd)[:, co * 128:(co + 1) * 128],
                            x_own[:, kd, :],
                            start=(kd == 0), stop=(kd == NKD - 1))
                for co in range(NKD):
                    nc.scalar.copy(qT_sb[:, co, :], ps_q[co][:])

                # wv loads into wq's (now dead) buffers
                w_sb["wv"] = [wxp.tile([128, 4, D], f32r, name=f"wv_{half}",
                                       tag=wtags["wv"][half])
                              for half in range(2)]
                for half in range(2):
                    nc.sync.dma_start(
                        out=w_sb["wv"][half],
                        in_=w_r["wv"][4 * half:4 * half + 4]
                        .rearrange("kd p c -> p kd c"))

                # ---- k projection (own tokens first, then halo)
                for th, xs in ((1, x_own), (0, x_halo)):
                    ps_k = [qps.tile([128, CHUNK], f32, name=f"psk{th}{co}",
                                     tag=f"ps{co}") for co in range(NKD)]
                    for kd in range(NKD):
                        for co in range(NKD):
                            nc.tensor.matmul(
                                ps_k[co][:],
                                wt("wk", kd)[:, co * 128:(co + 1) * 128],
                                xs[:, kd, :],
                                start=(kd == 0), stop=(kd == NKD - 1))
                    for co in range(NKD):
                        nc.scalar.copy(
                            kT_sb[:, co, th * CHUNK:(th + 1) * CHUNK],
                            ps_k[co][:])

                # ---- v projection, token-major: v[tt] = xT[:,tt].T @ wv
                for cv in range(2):
                    ps_v = [qps.tile([128, CHUNK], f32, name=f"psv{cv}{tt}",
                                     tag=f"ps{tt}") for tt in range(NKD)]
                    for kd in range(NKD):
                        for tt in range(NKD):
                            xs = x_halo if tt < 4 else x_own
                            tl = (tt % 4) * 128
                            nc.tensor.matmul(
                                ps_v[tt][:],
                                xs[:, kd, tl:tl + 128],
                                wt("wv", kd)[:, cv * CHUNK:(cv + 1) * CHUNK],
                                start=(kd == 0), stop=(kd == NKD - 1))
                    for tt in range(NKD):
                        # scatter 8 heads (cols of 64) into the 65-col slots
                        ps4 = ps_v[tt][:].rearrange(
                            "p (g par d) -> p g par d", par=2, d=HD)
                        g0 = cv * 4
                        nc.scalar.copy(
                            v_sb[:, tt, g0:g0 + 4, 0, 0:HD], ps4[:, :, 0, :])
                        nc.scalar.copy(
                            v_sb[:, tt, g0:g0 + 4, 1, 0:HD], ps4[:, :, 1, :])

            # ---- attention + output projection
            with tc.tile_pool(name="attb", bufs=1) as attbp, \
                 tc.tile_pool(name="nrm", bufs=2) as nrmp, \
                 tc.tile_pool(name="oev", bufs=2) as oevp:

                # attT pair layout: partitions 0:64 = even head dims,
                # 64:128 = odd head dims (DMA-shifted); slot hp = head pair
                attT = attbp.tile([128, NKD, CHUNK], f32r)

                # wo loads overlap attention (x/w space freed above)
                wo_sb = attbp.tile([128, NKD, D], f32r)
                for half in range(2):
                    nc.sync.dma_start(
                        out=wo_sb[:, 4 * half:4 * half + 4, :],
                        in_=wo_r[:, 4 * half:4 * half + 4, :])

                # persistent, manually double-buffered pt tiles; zero-pad
                # columns (ptA[384:512], ptD[0:128]) are written once here and
                # only ever read afterwards.
                ptA = [attbp.tile([128, 512], f32r, name=f"ptA{i}")
                       for i in range(2)]
                ptB = [attbp.tile([128, 896], f32r, name=f"ptB{i}")
                       for i in range(2)]
                ptC = [attbp.tile([128, 896], f32r, name=f"ptC{i}")
                       for i in range(2)]
                ptD = [attbp.tile([128, 512], f32r, name=f"ptD{i}")
                       for i in range(2)]
                for i in range(2):
                    nc.sync.dma_start(out=ptA[i][:, 384:512], in_=zpad[:, :])
                    nc.sync.dma_start(out=ptD[i][:, 0:128], in_=zpad[:, :])

                def rep_mask(plane):
                    # [128, 2, 128] view of one mask plane repeated twice
                    base = mask_sb[:, plane, :]
                    return bass.AP(tensor=base.tensor, offset=base.offset,
                                   ap=[base.ap[0], [0, 2], [1, 128]])

                def two_blocks(t, off, stride):
                    # [128, 2, 128] strided view: cols [off:off+128] and
                    # [off+stride:off+stride+128] of tile t
                    base = t[:, off:off + 128]
                    return bass.AP(tensor=base.tensor, offset=base.offset,
                                   ap=[base.ap[0], [stride, 2], [1, 128]])

                attention_scope = tc.tile_pool(name="ps_sc", bufs=1,
                                               space="PSUM")
                ps_sc = attention_scope.__enter__()
                at_scope = tc.tile_pool(name="ps_at", bufs=2, space="PSUM")
                ps_at = at_scope.__enter__()
                for h in range(H):
                    hp, po = h // 2, (h % 2) * 64
                    buf = h % 2
                    kt = kT_sb[po:po + 64, hp, :]
                    qt = qT_sb[po:po + 64, hp, :]

                    # ---- scores: 4 psum groups, query-aligned columns
                    scA = ps_sc.tile([128, 512], f32, tag="scA")
                    scB = ps_sc.tile([128, 1024], f32, tag="scB")
                    scC = ps_sc.tile([128, 1024], f32, tag="scC")
                    scD = ps_sc.tile([128, 512], f32, tag="scD")
                    # B: kb3 q[0:512) at cols 0:512, kb2 q[0:384) at 512:896
                    nc.tensor.matmul(scB[:, 0:512], kt[:, 384:512],
                                     qt[:, 0:512], start=True, stop=True)
                    nc.tensor.matmul(scB[:, 512:896], kt[:, 256:384],
                                     qt[:, 0:384], start=True, stop=True)
                    # C: kb4 q[0:512) at cols 0:512, kb5 q[128:512) at 512:896
                    nc.tensor.matmul(scC[:, 0:512], kt[:, 512:640],
                                     qt[:, 0:512], start=True, stop=True)
                    nc.tensor.matmul(scC[:, 512:896], kt[:, 640:768],
                                     qt[:, 128:512], start=True, stop=True)
                    # A: kb1 q[0:256) at cols 0:256, kb0 q[0:128) at 256:384
                    # (256:512 written, 384:512 is dead padding)
                    nc.tensor.matmul(scA[:, 0:256], kt[:, 128:256],
                                     qt[:, 0:256], start=True, stop=True)
                    nc.tensor.matmul(scA[:, 256:512], kt[:, 0:128],
                                     qt[:, 0:256], start=True, stop=True)
                    # D: kb7 q[384:512) at cols 128:256 (0:128 dead padding),
                    # kb6 q[256:512) at cols 256:512
                    nc.tensor.matmul(scD[:, 0:256], kt[:, 896:1024],
                                     qt[:, 256:512], start=True, stop=True)
                    nc.tensor.matmul(scD[:, 256:512], kt[:, 768:896],
                                     qt[:, 256:512], start=True, stop=True)

                    # ---- exp (one per group) + fused band masks
                    EXP = mybir.ActivationFunctionType.Exp
                    nc.scalar.activation(ptB[buf][:, 0:896], scB[:, 0:896],
                                         EXP, bias=kbias_sb[:, 1:2],
                                         scale=SCALE)
                    nc.vector.tensor_mul(two_blocks(ptB[buf], 384, 384),
                                         two_blocks(ptB[buf], 384, 384),
                                         rep_mask(0))
                    nc.scalar.activation(ptC[buf][:, 0:896], scC[:, 0:896],
                                         EXP, bias=kbias_sb[:, 2:3],
                                         scale=SCALE)
                    nc.vector.tensor_mul(two_blocks(ptC[buf], 0, 512),
                                         two_blocks(ptC[buf], 0, 512),
                                         rep_mask(1))
                    nc.scalar.activation(ptA[buf][:, 0:384], scA[:, 0:384],
                                         EXP, bias=kbias_sb[:, 0:1],
                                         scale=SCALE)
                    nc.vector.tensor_mul(ptA[buf][:, 128:384],
                                         ptA[buf][:, 128:384],
                                         rep_mask(0))
                    nc.scalar.activation(ptD[buf][:, 128:512], scD[:, 128:512],
                                         EXP, bias=kbias_sb[:, 3:4],
                                         scale=SCALE)
                    nc.vector.tensor_mul(ptD[buf][:, 128:384],
                                         ptD[buf][:, 128:384],
                                         rep_mask(1))

                    # ---- attV: v-stationary (65 cols: v dims + ones column)
                    att_ps = ps_at.tile([128, CHUNK], f32, tag="att")

                    def vst(kb):
                        return v_sb[:, kb, hp, h % 2, :]

                    nc.tensor.matmul(att_ps[0:65, 0:512], vst(3),
                                     ptB[buf][:, 0:512],
                                     start=True, stop=False)
                    nc.tensor.matmul(att_ps[0:65, 0:384], vst(2),
                                     ptB[buf][:, 512:896],
                                     start=False, stop=False)
                    nc.tensor.matmul(att_ps[0:65, 0:512], vst(4),
                                     ptC[buf][:, 0:512],
                                     start=False, stop=False)
                    nc.tensor.matmul(att_ps[0:65, 128:512], vst(5),
                                     ptC[buf][:, 512:896],
                                     start=False, stop=False)
                    nc.tensor.matmul(att_ps[0:65, 0:256], vst(1),
                                     ptA[buf][:, 0:256],
                                     start=False, stop=False)
                    nc.tensor.matmul(att_ps[0:65, 0:256], vst(0),
                                     ptA[buf][:, 256:512],
                                     start=False, stop=False)
                    nc.tensor.matmul(att_ps[0:65, 256:512], vst(7),
                                     ptD[buf][:, 0:256],
                                     start=False, stop=False)
                    nc.tensor.matmul(att_ps[0:65, 256:512], vst(6),
                                     ptD[buf][:, 256:512],
                                     start=False, stop=True)

                    # ---- normalize: recip of sums row, gpsimd partition
                    # broadcast (no DRAM bounce), psum-evicting multiply
                    rt = nrmp.tile([128, CHUNK], f32, tag="rt")
                    nc.vector.reciprocal(rt[64:65, :], att_ps[64:65, :])
                    bc = nrmp.tile([64, CHUNK], f32, tag="bc")
                    nc.gpsimd.partition_broadcast(bc[0:64, :], rt[64:65, :],
                                                  channels=64)
                    if h % 2 == 0:
                        nc.vector.tensor_mul(attT[0:64, hp, :],
                                             att_ps[0:64, :], bc[0:64, :])
                    else:
                        stag = nrmp.tile([64, CHUNK], f32r, tag="stag")
                        nc.vector.tensor_mul(stag[0:64, :],
                                             att_ps[0:64, :], bc[0:64, :])
                        # partition-shift 0:64 -> 64:128 (head-pair stacking)
                        nc.sync.dma_start(out=attT[64:128, hp, :],
                                          in_=stag[0:64, :])

                at_scope.__exit__(None, None, None)
                attention_scope.__exit__(None, None, None)

                # ---- output projection: 8 K=128 pair-matmuls per out tile
                with tc.tile_pool(name="ps_o", bufs=2, space="PSUM") as ps_o:
                    for eo in range(NKD):
                        ps = ps_o.tile([128, CHUNK], f32, tag="op")
                        for hp in range(NKD):
                            nc.tensor.matmul(
                                ps[:],
                                wo_sb[:, hp, eo * 128:(eo + 1) * 128],
                                attT[:, hp, :],
                                start=(hp == 0), stop=(hp == NKD - 1))
                        ot = oevp.tile([128, CHUNK], f32, tag="ot")
                        nc.scalar.copy(ot[:], ps[:])
                        nc.sync.dma_start(out=outT[eo * 128:(eo + 1) * 128, :],
                                          in_=ot[:])

    nc.compile()
    return nc


def _host_inputs(x, w_qkv, w_out):
    x = np.ascontiguousarray(np.asarray(x, dtype=np.float32))
    w_qkv = np.ascontiguousarray(np.asarray(w_qkv, dtype=np.float32))
    w_out = np.ascontiguousarray(np.asarray(w_out, dtype=np.float32))

    wq = np.ascontiguousarray(w_qkv[:, 0:D])
    wk = np.ascontiguousarray(w_qkv[:, D:2 * D])
    wv = np.ascontiguousarray(w_qkv[:, 2 * D:3 * D])

    r = np.arange(128)[:, None]
    c = np.arange(128)[None, :]
    mask = np.zeros((128, 2, 128), dtype=np.float32)
    mask[:, 0, :] = (r > c).astype(np.float32)   # halo diag blocks
    mask[:, 1, :] = (r <= c).astype(np.float32)  # own diag blocks
    vones = np.ones((128, NKD, H), dtype=np.float32)
    zpad = np.zeros((128, 128), dtype=np.float32)

    in_maps = []
    for core in range(NCORES):
        b, qc = divmod(core, 4)
        q0 = qc * CHUNK
        xa = np.zeros((TOK, D), dtype=np.float32)
        lo = max(0, q0 - CHUNK)
        xa[CHUNK - (q0 - lo):] = x[b, lo:q0 + CHUNK]
        kb_bias = np.zeros((128, 4), dtype=np.float32)
        if qc == 0:
            kb_bias[:, 0:2] = -250.0  # groups A,B cover the (zero) halo keys
        in_maps.append({
            "xT": np.ascontiguousarray(xa.T),
            "wq": wq, "wk": wk, "wv": wv, "wo": w_out,
            "mask": mask, "kbias": kb_bias, "vones": vones, "zpad": zpad,
        })
    return in_maps


def kernel(x, w_qkv, w_out):
    global _BUILT
    if _BUILT is None:
        _BUILT = _build()
    from concourse.bass_utils import run_bass_kernel_spmd

    in_maps = _host_inputs(x, w_qkv, w_out)
    res = run_bass_kernel_spmd(_BUILT, in_maps, core_ids=list(range(NCORES)))
    out = np.empty((B, T, D), dtype=np.float32)
    for core in range(NCORES):
        b, qc = divmod(core, 4)
        out[b, qc * CHUNK:(qc + 1) * CHUNK, :] = res.results[core]["outT"].T
    return out


# revision 13
# speedup vs baseline: 1.4383x; 1.2339x over previous
"""Causal sliding-window attention (B=2, T=2048, D=1024, H=16, W=512) on 8 trn2 cores.

Sequence-parallel sharding: each core owns 512 consecutive tokens of one batch
and recomputes the 512-token halo k/v locally (no collectives). All compute is
feature-major (tokens on the matmul free dim) in float32r.

v3: engine-balanced pipeline, DMA-stream-ordered startup.
  - DMA transfers are serial (~360 B/ns) in the machine, so transfers are
    issued in first-use order at per-kd granularity for the q-proj gate:
    x_own/wq interleaved, then wv, x_halo, wk, wo. Phases run q -> v -> k so
    k's staggered co-outer evictions feed straight into attention.
  - attention: 8 key-tiles per head packed into 4 psum score groups
    (A=kb1|kb0, B=kb3|kb2, C=kb4|kb5, D=Z|kb7|kb6) with query-aligned column
    layouts; one exp per group (4 ACT instrs/head), one fused band-mask
    multiply per group (strided/repeated APs, 4 DVE instrs/head); padded attV
    windows read permanently zeroed pt columns (Pool memset on a bitcast f32
    view; f32r cannot be memset directly).
  - softmax: a ones column rides in the attV stationary; even heads use
    [v|ones] -> psum rows 0:65, odd heads [zeros63|ones|v] -> rows 63:128, so
    the normalized pair lands on partitions 0:64 / 64:128 of one attT tile
    with no cross-partition copy. DVE reciprocal -> gpsimd partition_broadcast
    -> DVE multiply, all partition-base aligned.
  - out-proj: head-pair contraction K=128, 8 matmuls per output tile; wo rows
    for pair hp are the contiguous block [128*hp, 128*hp+128).
"""
import sys

sys.path.insert(0, "/opt/trn_rl_repo")

import numpy as np

B, T, D = 2, 2048, 1024
H, HD, W = 16, 64, 512
NCORES = 8
CHUNK = 512  # own tokens per core
TOK = 2 * CHUNK  # halo + own
NKD = D // 128  # 8 contraction tiles
SCALE = HD ** -0.5
VW = 193  # v_sb stationary cols per (key-tile, head-pair): [v|ones] + [z|1|v]

_BUILT = None


def _build():
    import concourse.bass as bass
    import concourse.tile as tile
    from concourse import mybir, bacc

    f32 = mybir.dt.float32
    f32r = mybir.dt.float32r

    nc = bacc.Bacc("TRN2", target_bir_lowering=False, debug=False,
                   num_devices=NCORES)
    xT = nc.dram_tensor("xT", [D, TOK], f32r, kind="ExternalInput")
    wq = nc.dram_tensor("wq", [D, D], f32r, kind="ExternalInput")
    wk = nc.dram_tensor("wk", [D, D], f32r, kind="ExternalInput")
    wv = nc.dram_tensor("wv", [D, D], f32r, kind="ExternalInput")
    wo = nc.dram_tensor("wo", [D, D], f32r, kind="ExternalInput")
    # [:, 0, :] = strict-lower-tri (halo diag blocks), [:, 1, :] = upper-incl
    mask = nc.dram_tensor("mask", [128, 2, 128], f32, kind="ExternalInput")
    # exp bias per score group (A,B = halo: -250 on chunk-0 cores; C,D = 0)
    kbias = nc.dram_tensor("kbias", [128, 4], f32, kind="ExternalInput")
    outT = nc.dram_tensor("outT", [D, CHUNK], f32, kind="ExternalOutput")

    xT_r = xT.rearrange("(kd p) t -> kd p t", p=128)
    w_r = {n: w.rearrange("(kd p) c -> kd p c", p=128)
           for n, w in (("wq", wq), ("wk", wk), ("wv", wv))}
    # wo with head-pair rows contiguous: partition p of pair hp = wo row
    # 128*hp + p (head 2hp dims at p<64, head 2hp+1 dims at p>=64)
    wo_r = wo.rearrange("(hp p) e -> p hp e", p=128)

    with tile.TileContext(nc) as tc:
        with tc.tile_pool(name="const", bufs=1) as constp, \
             tc.tile_pool(name="qkv", bufs=1) as qkvp:

            mask_sb = constp.tile([128, 2, 128], f32)
            nc.sync.dma_start(out=mask_sb, in_=mask[:, :, :])
            kbias_sb = constp.tile([128, 4], f32)
            nc.sync.dma_start(out=kbias_sb, in_=kbias[:, :])

            # ---- persistent qkv buffers (feature-major q/k, token-major v)
            qT_sb = qkvp.tile([128, NKD, CHUNK], f32r)   # q dims x own tokens
            kT_sb = qkvp.tile([128, NKD, TOK], f32r)     # k dims x keys
            # v stationaries per (key-tile, head-pair): cols 0:64 = even v,
            # 64 = even ones, 65:128 = zeros, 128 = odd ones, 129:193 = odd v
            # (odd slice [65:193] puts odd sums at psum row 63, v at 64:128)
            v_sb = qkvp.tile([128, NKD, H // 2, VW], f32r)
            nc.gpsimd.memset(v_sb[:, :, :, 65:128].bitcast(f32), 0.0)
            nc.gpsimd.memset(v_sb[:, :, :, 64:65].bitcast(f32), 1.0)
            nc.gpsimd.memset(v_sb[:, :, :, 128:129].bitcast(f32), 1.0)

            with tc.tile_pool(name="wx", bufs=1) as wxp, \
                 tc.tile_pool(name="qkv_ps", bufs=1, space="PSUM") as qps:
                x_own = wxp.tile([128, NKD, CHUNK], f32r)
                x_halo = wxp.tile([128, NKD, CHUNK], f32r)
                # wk reuses wq's buffers (k-proj runs last; its DMAs wait for
                # q-proj's final reads); wv gets its own, prefetchable pair.
                w_sb = {wn: [wxp.tile([128, 4, D], f32r, name=f"{wn}_{half}",
                             tag=f"{tg}{half}")
                             for half in range(2)]
                        for wn, tg in (("wq", "wqk"), ("wv", "wv"))}

                # DMA issue order == first-use order (transfers are serial,
                # ~360 B/ns): per-kd x_own/wq pairs gate q-proj start at ~3us;
                # wv and x_halo stream behind during q/v compute; wk last.
                for kd in range(NKD):
                    nc.sync.dma_start(out=x_own[:, kd, :],
                                      in_=xT_r[kd, :, CHUNK:TOK])
                    nc.sync.dma_start(out=w_sb["wq"][kd // 4][:, kd % 4, :],
                                      in_=w_r["wq"][kd])
                for kd in range(NKD):
                    nc.sync.dma_start(out=w_sb["wv"][kd // 4][:, kd % 4, :],
                                      in_=w_r["wv"][kd])
                for kd in range(NKD):
                    nc.sync.dma_start(out=x_halo[:, kd, :],
                                      in_=xT_r[kd, :, 0:CHUNK])

                def wt(wn, kd):
                    return w_sb[wn][kd // 4][:, kd % 4, :]

                # ---- q projection, kd-outer: 8 parallel psum chains
                ps_q = [qps.tile([128, CHUNK], f32, name=f"psq{co}",
                                 tag=f"ps{co}") for co in range(NKD)]
                for kd in range(NKD):
                    for co in range(NKD):
                        nc.tensor.matmul(
                            ps_q[co][:],
                            wt("wq", kd)[:, co * 128:(co + 1) * 128],
                            x_own[:, kd, :],
                            start=(kd == 0), stop=(kd == NKD - 1))
                for co in range(NKD):
                    nc.scalar.copy(qT_sb[:, co, :], ps_q[co][:])

                # wk loads into wq's (now dead) buffers
                w_sb["wk"] = [wxp.tile([128, 4, D], f32r, name=f"wk_{half}",
                                       tag=f"wqk{half}")
                              for half in range(2)]
                for kd in range(NKD):
                    nc.sync.dma_start(out=w_sb["wk"][kd // 4][:, kd % 4, :],
                                      in_=w_r["wk"][kd])

                # ---- v projection (own token tiles first, halo second),
                # kd-outer groups of 8 chains; v[tt] = xT[:,tt].T @ wv
                for grp in (range(4, 8), range(0, 4)):
                    ps_v = {(tt, cv): qps.tile(
                        [128, CHUNK], f32, name=f"psv{cv}{tt}",
                        tag=f"ps{(tt % 4) * 2 + cv}")
                        for tt in grp for cv in range(2)}
                    for kd in range(NKD):
                        for tt in grp:
                            xs = x_halo if tt < 4 else x_own
                            tl = (tt % 4) * 128
                            for cv in range(2):
                                nc.tensor.matmul(
                                    ps_v[tt, cv][:],
                                    xs[:, kd, tl:tl + 128],
                                    wt("wv", kd)[:, cv * CHUNK:(cv + 1) * CHUNK],
                                    start=(kd == 0), stop=(kd == NKD - 1))
                    for tt in grp:
                        for cv in range(2):
                            # scatter 8 heads (cols of 64) into v stationaries
                            ps4 = ps_v[tt, cv][:].rearrange(
                                "p (g par d) -> p g par d", par=2, d=HD)
                            g0 = cv * 4
                            nc.scalar.copy(
                                v_sb[:, tt, g0:g0 + 4, 0:HD], ps4[:, :, 0, :])
                            nc.scalar.copy(
                                v_sb[:, tt, g0:g0 + 4, 129:VW], ps4[:, :, 1, :])

                # ---- k projection: own tokens kd-outer (wk still arriving),
                # then halo co-outer so evictions stagger into attention
                ps_k = [qps.tile([128, CHUNK], f32, name=f"psk1{co}",
                                 tag=f"ps{co}") for co in range(NKD)]
                for kd in range(NKD):
                    for co in range(NKD):
                        nc.tensor.matmul(
                            ps_k[co][:],
                            wt("wk", kd)[:, co * 128:(co + 1) * 128],
                            x_own[:, kd, :],
                            start=(kd == 0), stop=(kd == NKD - 1))
                for co in range(NKD):
                    nc.scalar.copy(kT_sb[:, co, CHUNK:TOK], ps_k[co][:])
                for co in range(NKD):
                    ps = qps.tile([128, CHUNK], f32, name=f"psk0{co}",
                                  tag=f"ps{co}")
                    for kd in range(NKD):
                        nc.tensor.matmul(
                            ps[:],
                            wt("wk", kd)[:, co * 128:(co + 1) * 128],
                            x_halo[:, kd, :],
                            start=(kd == 0), stop=(kd == NKD - 1))
                    nc.scalar.copy(kT_sb[:, co, 0:CHUNK], ps[:])

            # ---- attention + output projection
            with tc.tile_pool(name="attb", bufs=1) as attbp, \
                 tc.tile_pool(name="nrm", bufs=2) as nrmp, \
                 tc.tile_pool(name="oev", bufs=2) as oevp:

                # attT pair layout: partitions 0:64 = even head dims,
                # 64:128 = odd head dims; slot hp = head pair
                attT = attbp.tile([128, NKD, CHUNK], f32r)

                # wo loads overlap attention (x/w space freed above)
                wo_sb = attbp.tile([128, NKD, D], f32r)
                for half in range(2):
                    nc.sync.dma_start(
                        out=wo_sb[:, 4 * half:4 * half + 4, :],
                        in_=wo_r[:, 4 * half:4 * half + 4, :])

                # persistent, manually double-buffered pt tiles; zero-pad
                # columns (ptA[384:512], ptD[0:128]) are written once and
                # only ever read afterwards.
                ptA = [attbp.tile([128, 512], f32r, name=f"ptA{i}")
                       for i in range(2)]
                ptB = [attbp.tile([128, 896], f32r, name=f"ptB{i}")
                       for i in range(2)]
                ptC = [attbp.tile([128, 896], f32r, name=f"ptC{i}")
                       for i in range(2)]
                ptD = [attbp.tile([128, 512], f32r, name=f"ptD{i}")
                       for i in range(2)]
                for i in range(2):
                    nc.vector.memset(ptA[i][:, 384:512].bitcast(f32), 0.0)
                    nc.vector.memset(ptD[i][:, 0:128].bitcast(f32), 0.0)

                def rep_mask(plane):
                    # [128, 2, 128] view of one mask plane repeated twice
                    base = mask_sb[:, plane, :]
                    return bass.AP(tensor=base.tensor, offset=base.offset,
                                   ap=[list(base.ap[0]), [0, 2], [1, 128]])

                def two_blocks(t, off, stride):
                    # [128, 2, 128] strided view: cols [off:off+128] and
                    # [off+stride:off+stride+128] of tile t
                    base = t[:, off:off + 128]
                    return bass.AP(tensor=base.tensor, offset=base.offset,
                                   ap=[list(base.ap[0]), [stride, 2],
                                       [1, 128]])

                attention_scope = tc.tile_pool(name="ps_sc", bufs=1,
                                               space="PSUM")
                ps_sc = attention_scope.__enter__()
                at_scope = tc.tile_pool(name="ps_at", bufs=2, space="PSUM")
                ps_at = at_scope.__enter__()
                for h in range(H):
                    hp, odd = h // 2, h % 2
                    po = odd * 64
                    buf = h % 2
                    kt = kT_sb[po:po + 64, hp, :]
                    qt = qT_sb[po:po + 64, hp, :]

                    # ---- scores: 4 psum groups, query-aligned columns
                    scA = ps_sc.tile([128, 512], f32, tag="scA")
                    scB = ps_sc.tile([128, 1024], f32, tag="scB")
                    scC = ps_sc.tile([128, 1024], f32, tag="scC")
                    scD = ps_sc.tile([128, 512], f32, tag="scD")
                    # B: kb3 q[0:512) at cols 0:512, kb2 q[0:384) at 512:896
                    nc.tensor.matmul(scB[:, 0:512], kt[:, 384:512],
                                     qt[:, 0:512], start=True, stop=True)
                    nc.tensor.matmul(scB[:, 512:896], kt[:, 256:384],
                                     qt[:, 0:384], start=True, stop=True)
                    # C: kb4 q[0:512) at cols 0:512, kb5 q[128:512) at 512:896
                    nc.tensor.matmul(scC[:, 0:512], kt[:, 512:640],
                                     qt[:, 0:512], start=True, stop=True)
                    nc.tensor.matmul(scC[:, 512:896], kt[:, 640:768],
                                     qt[:, 128:512], start=True, stop=True)
                    # A: kb1 q[0:256) at cols 0:256, kb0 q[0:128) at 256:384
                    # (256:512 written, 384:512 is dead padding)
                    nc.tensor.matmul(scA[:, 0:256], kt[:, 128:256],
                                     qt[:, 0:256], start=True, stop=True)
                    nc.tensor.matmul(scA[:, 256:512], kt[:, 0:128],
                                     qt[:, 0:256], start=True, stop=True)
                    # D: kb7 q[384:512) at cols 128:256 (0:256 written, real
                    # at 128:256), kb6 q[256:512) at cols 256:512
                    nc.tensor.matmul(scD[:, 0:256], kt[:, 896:1024],
                                     qt[:, 256:512], start=True, stop=True)
                    nc.tensor.matmul(scD[:, 256:512], kt[:, 768:896],
                                     qt[:, 256:512], start=True, stop=True)

                    # ---- exp (one per group) + fused band masks
                    EXP = mybir.ActivationFunctionType.Exp
                    nc.scalar.activation(ptB[buf][:, 0:896], scB[:, 0:896],
                                         EXP, bias=kbias_sb[:, 1:2],
                                         scale=SCALE)
                    nc.vector.tensor_mul(two_blocks(ptB[buf], 384, 384),
                                         two_blocks(ptB[buf], 384, 384),
                                         rep_mask(0))
                    nc.scalar.activation(ptC[buf][:, 0:896], scC[:, 0:896],
                                         EXP, bias=kbias_sb[:, 2:3],
                                         scale=SCALE)
                    nc.vector.tensor_mul(two_blocks(ptC[buf], 0, 512),
                                         two_blocks(ptC[buf], 0, 512),
                                         rep_mask(1))
                    nc.scalar.activation(ptA[buf][:, 0:384], scA[:, 0:384],
                                         EXP, bias=kbias_sb[:, 0:1],
                                         scale=SCALE)
                    nc.vector.tensor_mul(ptA[buf][:, 128:384],
                                         ptA[buf][:, 128:384],
                                         rep_mask(0))
                    nc.scalar.activation(ptD[buf][:, 128:512], scD[:, 128:512],
                                         EXP, bias=kbias_sb[:, 3:4],
                                         scale=SCALE)
                    nc.vector.tensor_mul(ptD[buf][:, 128:384],
                                         ptD[buf][:, 128:384],
                                         rep_mask(1))

                    # ---- attV: even heads [v|ones] -> psum rows 0:65 (sums
                    # at 64); odd heads [zeros63|ones|v] -> rows 0:128 (sums
                    # at 63, v at 64:128; rows 0:63 accumulate zeros)
                    att_ps = ps_at.tile([128, CHUNK], f32, tag="att")
                    nr = 128 if odd else 65

                    def vst(kb):
                        if odd:
                            return v_sb[:, kb, hp, 65:VW]
                        return v_sb[:, kb, hp, 0:65]

                    nc.tensor.matmul(att_ps[0:nr, 0:512], vst(3),
                                     ptB[buf][:, 0:512],
                                     start=True, stop=False)
                    nc.tensor.matmul(att_ps[0:nr, 0:384], vst(2),
                                     ptB[buf][:, 512:896],
                                     start=False, stop=False)
                    nc.tensor.matmul(att_ps[0:nr, 0:512], vst(4),
                                     ptC[buf][:, 0:512],
                                     start=False, stop=False)
                    nc.tensor.matmul(att_ps[0:nr, 128:512], vst(5),
                                     ptC[buf][:, 512:896],
                                     start=False, stop=False)
                    nc.tensor.matmul(att_ps[0:nr, 0:256], vst(1),
                                     ptA[buf][:, 0:256],
                                     start=False, stop=False)
                    nc.tensor.matmul(att_ps[0:nr, 0:256], vst(0),
                                     ptA[buf][:, 256:512],
                                     start=False, stop=False)
                    nc.tensor.matmul(att_ps[0:nr, 256:512], vst(7),
                                     ptD[buf][:, 0:256],
                                     start=False, stop=False)
                    nc.tensor.matmul(att_ps[0:nr, 256:512], vst(6),
                                     ptD[buf][:, 256:512],
                                     start=False, stop=True)

                    # ---- normalize: recip of sums row, gpsimd partition
                    # broadcast, psum-evicting multiply; all partition-aligned
                    srow = 63 if odd else 64
                    rt = nrmp.tile([128, CHUNK], f32, tag="rt")
                    nc.vector.reciprocal(rt[srow:srow + 1, :],
                                         att_ps[srow:srow + 1, :])
                    bc = nrmp.tile([128, CHUNK], f32, tag="bc")
                    nc.gpsimd.partition_broadcast(bc[po:po + 64, :],
                                                  rt[srow:srow + 1, :],
                                                  channels=64)
                    nc.vector.tensor_mul(attT[po:po + 64, hp, :],
                                         att_ps[po:po + 64, :],
                                         bc[po:po + 64, :])

                at_scope.__exit__(None, None, None)
                attention_scope.__exit__(None, None, None)

                # ---- output projection: 8 K=128 pair-matmuls per out tile
                with tc.tile_pool(name="ps_o", bufs=3, space="PSUM") as ps_o:
                    for eo in range(NKD):
                        ps = ps_o.tile([128, CHUNK], f32, tag="op")
                        for hp in range(NKD):
                            nc.tensor.matmul(
                                ps[:],
                                wo_sb[:, hp, eo * 128:(eo + 1) * 128],
                                attT[:, hp, :],
                                start=(hp == 0), stop=(hp == NKD - 1))
                        ot = oevp.tile([128, CHUNK], f32, tag="ot")
                        nc.scalar.copy(ot[:], ps[:])
                        nc.sync.dma_start(out=outT[eo * 128:(eo + 1) * 128, :],
                                          in_=ot[:])

    nc.compile()
    return nc


def _host_inputs(x, w_qkv, w_out):
    x = np.ascontiguousarray(np.asarray(x, dtype=np.float32))
    w_qkv = np.ascontiguousarray(np.asarray(w_qkv, dtype=np.float32))
    w_out = np.ascontiguousarray(np.asarray(w_out, dtype=np.float32))

    wq = np.ascontiguousarray(w_qkv[:, 0:D])
    wk = np.ascontiguousarray(w_qkv[:, D:2 * D])
    wv = np.ascontiguousarray(w_qkv[:, 2 * D:3 * D])

    r = np.arange(128)[:, None]
    c = np.arange(128)[None, :]
    mask = np.zeros((128, 2, 128), dtype=np.float32)
    mask[:, 0, :] = (r > c).astype(np.float32)   # halo diag blocks
    mask[:, 1, :] = (r <= c).astype(np.float32)  # own diag blocks

    in_maps = []
    for core in range(NCORES):
        b, qc = divmod(core, 4)
        q0 = qc * CHUNK
        xa = np.zeros((TOK, D), dtype=np.float32)
        lo = max(0, q0 - CHUNK)
        xa[CHUNK - (q0 - lo):] = x[b, lo:q0 + CHUNK]
        kb_bias = np.zeros((128, 4), dtype=np.float32)
        if qc == 0:
            kb_bias[:, 0:2] = -250.0  # groups A,B cover the (zero) halo keys
        in_maps.append({
            "xT": np.ascontiguousarray(xa.T),
            "wq": wq, "wk": wk, "wv": wv, "wo": w_out,
            "mask": mask, "kbias": kb_bias,
        })
    return in_maps


def kernel(x, w_qkv, w_out):
    global _BUILT
    if _BUILT is None:
        _BUILT = _build()
    from concourse.bass_utils import run_bass_kernel_spmd

    in_maps = _host_inputs(x, w_qkv, w_out)
    res = run_bass_kernel_spmd(_BUILT, in_maps, core_ids=list(range(NCORES)))
    out = np.empty((B, T, D), dtype=np.float32)
    for core in range(NCORES):
        b, qc = divmod(core, 4)
        out[b, qc * CHUNK:(qc + 1) * CHUNK, :] = res.results[core]["outT"].T
    return out
